# revision 39
# baseline (speedup 1.0000x reference)
"""Bass/Trainium2 kernel for nn_Encoder_47210280517649 (Pyraformer encoder).

Data-parallel over batch (B=16 -> 2 per core x 8 cores). bf16 compute:
bf16 x-master + bf16 weights (double-buffered), windowed/packed S,AV
attention matmuls with region-scoped PSUM accumulation, per-head softmax
normalization via partition_broadcast, 2-op LayerNorm tails with bias
folding (rstd via exp(-0.5*ln(var+eps)) so Act stays in one table set per
slot), slot-paired generator interleaving, CSCM with the token-embedding
-> down-projection matmul folded host-side.
"""
import os
import sys
import contextlib
import numpy as np

try:
    import concourse  # noqa: F401
except ImportError:
    for _p in ("/opt/trn_rl_repo", "/root/.axon_site/_ro/trn_rl_repo"):
        sys.path.insert(0, _p)

import concourse.bass as bass  # noqa: E402
import concourse.mybir as mybir  # noqa: E402
import concourse.tile as tile  # noqa: E402
from concourse import bacc  # noqa: E402
from concourse.bass_utils import run_bass_kernel_spmd  # noqa: E402
from concourse.alu_op_type import AluOpType  # noqa: E402

dt = mybir.dt
F32, F32R, BF16 = dt.float32, dt.float32r, dt.bfloat16
Act = mybir.ActivationFunctionType
AO = AluOpType

N_CORES = 8
NB = 16
LB = 2
SEQ = 512
DM = 512
L = 680
NH = 8
DFF = 2048
NL = 4
HW = 340
KT = [(0, 128), (128, 256), (256, 384), (384, 512), (512, 640), (640, 680)]


# ---------------- host-side attention plan ----------------
def _attn_plan(attn_mask):
    """Windows per (r,hf) packed into <=512-col psum groups; AV segments.

    Returns:
      win[(r,hf)]   = [(q0, w, g, poff)]  S/exp windows + packed offsets
      grp_w[(hf,g)] = total packed width of group g
      segs[hf]      = [(q0, w, [(r, g, poff_seg), ...])] AV segments
    """
    keep = ~np.asarray(attn_mask)
    win, grp_w = {}, {}
    for hf in range(2):
        for r, (a, b) in enumerate(KT):
            cols = keep[a:b, hf * HW:(hf + 1) * HW].any(axis=0)
            if not cols.any():
                continue
            idx = np.where(cols)[0]
            runs = []
            s = p = int(idx[0])
            for i in idx[1:]:
                if i - p < 32:
                    p = int(i)
                else:
                    runs.append((s, p - s + 1))
                    s = p = int(i)
            runs.append((s, p - s + 1))
            lst = []
            for (q0, w) in runs:
                for g in range(8):
                    u = grp_w.get((hf, g), 0)
                    if u + w <= 512:
                        lst.append((q0, w, g, u))
                        grp_w[(hf, g)] = u + w
                        break
                else:
                    raise AssertionError("window packing failed")
            win[(r, hf)] = lst
    segs = {}
    for hf in range(2):
        edges = {0, HW}
        for (r, h), lst in win.items():
            if h != hf:
                continue
            for (q0, w, g, u) in lst:
                edges.add(q0)
                edges.add(q0 + w)
        edges = sorted(edges)
        out = []
        for i in range(len(edges) - 1):
            s, e = edges[i], edges[i + 1]
            contrib = []
            for r in range(6):
                for (q0, w, g, u) in win.get((r, hf), []):
                    if q0 <= s and e <= q0 + w:
                        contrib.append((r, g, u + (s - q0)))
            if contrib:
                out.append((s, e - s, contrib))
        segs[hf] = out
    return win, grp_w, segs


def _m01_pack(attn_mask, win, grp_w):
    """0/1 keep mask packed to group layout: [128, TOT] + offsets."""
    keep = ~np.asarray(attn_mask)
    ng = {hf: max(g for (h, g) in grp_w if h == hf) + 1 for hf in range(2)}
    off, offs = 0, {}
    for hf in range(2):
        for g in range(ng[hf]):
            offs[(hf, g)] = off
            off += grp_w[(hf, g)]
    m01 = np.zeros((128, off), np.float32)
    for (r, hf), lst in win.items():
        a, b = KT[r]
        for (q0, w, g, u) in lst:
            m01[0:b - a, offs[(hf, g)] + u: offs[(hf, g)] + u + w] = \
                keep[a:b, hf * HW + q0: hf * HW + q0 + w]
    return m01, offs, off, ng


# ---------------- device program ----------------
def _build_program(plan):
    win, grp_w, segs, m01w, m01offs, ng = plan
    nc = bacc.Bacc("TRN2", target_bir_lowering=False, debug=False,
                   num_devices=N_CORES)

    def din(name, shape, dtp):
        return nc.dram_tensor(name, shape, dtp, kind="ExternalInput")

    D = dict(
        x25p=din("x25p", [NB // 2, 25, 2 * SEQ], F32R),
        wp=din("wp", [25, DM + 128], F32R),
        cembP=din("cembP", [128, 4 * SEQ], BF16),
        dcorr=din("dcorr", [128, SEQ], F32),
        ckP=din("ckP", [128, 1536], BF16),
        upw=din("upw", [128, DM], BF16),
        m01=din("m01", [128, max(m01w, 1)], BF16),
        wq=din("wq", [NL, DM, DM], BF16),
        wk=din("wk", [NL, DM, DM], BF16),
        wv=din("wv", [NL, DM, DM], BF16),
        wo=din("wo", [NL, DM, DM], BF16),
        w1=din("w1", [NL, DM, DFF], BF16),
        w2=din("w2", [NL, DFF, DM], BF16),
        bvB=din("bvB", [NL, 128, DM], BF16),
        tab_bq=din("tab_bq", [128, NL * 4], F32),
        tab_bk=din("tab_bk", [128, NL * 4], F32),
        tab_bo=din("tab_bo", [128, NL * 4], F32),
        tab_b2=din("tab_b2", [128, NL * 4], F32),
        tab_b1=din("tab_b1", [128, NL * 16], F32),
        tab_g1=din("tab_g1", [128, NL * 4], F32),
        tab_g2=din("tab_g2", [128, NL * 4], F32),
        tab_gcn=din("tab_gcn", [128, 4], F32),
        tab_cs=din("tab_cs", [128, 16], F32),
        ones128_d=din("ones128_d", [128, 1], F32R),
        ones1_d=din("ones1_d", [1, 128], F32R),
        zrow_d=din("zrow_d", [1, 512], F32R),
        out=nc.dram_tensor("out", [LB, 4, 128, L], F32R,
                           kind="ExternalOutput"),
    )
    if os.environ.get("KDBG"):
        D["dbg0"] = nc.dram_tensor("dbg0", [4, 128, L], BF16,
                                   kind="ExternalOutput")
        D["dbg1"] = nc.dram_tensor("dbg1", [4, 128, L], BF16,
                                   kind="ExternalOutput")
        D["dbg2"] = nc.dram_tensor("dbg2", [4, 128, L], BF16,
                                   kind="ExternalOutput")
    with tile.TileContext(nc) as tc:
        with nc.allow_low_precision(reason="bf16 compute by design"):
            _emit(nc, tc, D, win, grp_w, segs, m01offs, ng)
    _install_act_table_pass(nc)
    nc.compile()
    return nc


def _install_act_table_pass(nc):
    """Replace Bacc.insert_act_table_loads with a lookahead-greedy chooser.

    The stock pass picks the first table set containing each function
    (exp->0, ln->5), so exp/ln alternation reloads tables constantly
    (~1.3us each).  Choosing the set that covers the longest upcoming run
    (e.g. set 6 = ln+exp+identity+copy+square) drops ~100 loads.
    """
    from concourse.hw_specs import get_activation_tables

    def pass_(self=nc):
        tables = list(get_activation_tables(nc.m.arch).items())
        sets = [funcs for _nm, funcs in tables]
        for blk in nc.main_func.blocks:
            funcs = [(j, i.func) for j, i in enumerate(blk.instructions)
                     if isinstance(i, mybir.InstActivation)]
            if not funcs:
                continue
            flist = [f for _j, f in funcs]
            inserts = []
            cur = None
            for k, (j, f) in enumerate(funcs):
                if cur is not None and f in sets[cur]:
                    continue
                best, bestlen = None, -1
                order = [6, 10] + [i for i in range(len(sets))
                                   if i not in (6, 10)]
                for si in order:
                    s = sets[si]
                    if f not in s:
                        continue
                    n = 0
                    for nx in flist[k:]:
                        if nx in s:
                            n += 1
                        else:
                            break
                    if n > bestlen:
                        best, bestlen = si, n
                cur = best
                inserts.append((j, best))
            for j, si in reversed(inserts):
                ld = mybir.InstLoadActFuncSet(
                    name=nc.get_next_instruction_name(),
                    act_func_set_id=si, ins=[], outs=[])
                ld.engine = blk.instructions[j].engine
                nc.register_instruction(ld)
                blk.instructions.insert(j, ld)

    nc.insert_act_table_loads = pass_


def _drive(*gens, delay=None):
    """Round-robin emission. delay[i] = rounds before gen i joins, so
    chunks whose deps complete late don't head-of-line block engine queues.
    """
    pend = [(g, 0 if delay is None else delay[i])
            for i, g in enumerate(gens) if g is not None]
    act = []
    rnd = 0
    while pend or act:
        for it in list(pend):
            if rnd >= it[1]:
                act.append(it[0])
                pend.remove(it)
        rnd += 1
        if not act:
            continue
        for g in list(act):
            try:
                next(g)
            except StopIteration:
                act.remove(g)


def _emit(nc, tc, D, win, grp_w, segs, m01offs, ng):
    v, sc, te, sy, gp = nc.vector, nc.scalar, nc.tensor, nc.sync, nc.gpsimd
    ctx = contextlib.ExitStack()
    with ctx:
        persist = ctx.enter_context(tc.tile_pool(name="persist", bufs=1))
        small = ctx.enter_context(tc.tile_pool(name="small", bufs=1))
        wpool = ctx.enter_context(tc.tile_pool(name="wts", bufs=1))
        pp = ctx.enter_context(tc.tile_pool(name="ps", bufs=1, space="PSUM"))

        # ---- manual PSUM banks (8 x [128,512] f32 = full PSUM) ----
        qb = [pp.tile([128, 512], F32, name=f"qb{i}") for i in range(2)]
        sb = [pp.tile([128, 512], F32, name=f"sb{i}") for i in range(2)]
        pob = [pp.tile([128, 512], F32, name=f"pob{i}") for i in range(2)]
        lnb = pp.tile([128, 512], F32, name="lnb")
        ffnb = pp.tile([128, 512], F32, name="ffnb")
        qb_i = [0]
        pob_i = [0]

        def qb_next():
            qb_i[0] += 1
            return qb[qb_i[0] % 2]

        def pob_next():
            pob_i[0] += 1
            return pob[pob_i[0] % 2]

        # ---- persistent tiles ----
        ones128f = persist.tile([128, 1], F32R, name="ones128f")
        sy.dma_start(ones128f[:], D["ones128_d"].ap())
        ones128b = persist.tile([128, 1], BF16, name="ones128b")
        v.memset(ones128b[:], 1.0)
        ones1 = persist.tile([1, 128], F32R, name="ones1")
        sy.dma_start(ones1[:], D["ones1_d"].ap())
        epsT = persist.tile([1, 1], F32, name="epsT")
        v.memset(epsT[:], 1e-5)
        zrow = persist.tile([1, 512], F32R, name="zrow")
        sy.dma_start(zrow[:], D["zrow_d"].ap())
        # S banks can be read by exp over never-matmul'd cells: bound them
        # with a zero outer-product (DVE memset cannot target PSUM).
        te.matmul(sb[0][:, 0:512], ones1[:], zrow[:], start=True, stop=True)
        te.matmul(sb[1][:, 0:512], ones1[:], zrow[:], start=True, stop=True)

        eps128 = persist.tile([128, 1], F32, name="eps128")
        v.memset(eps128[:], 1e-5)

        m01_sb = persist.tile([128, D["m01"].shape[1]], BF16, name="m01_sb")
        tabs = {}
        for nm in ("tab_bq", "tab_bk", "tab_bo", "tab_b2", "tab_b1",
                   "tab_g1", "tab_g2", "tab_gcn", "tab_cs"):
            tabs[nm] = persist.tile(list(D[nm].shape), F32, name=nm + "_sb")
        sy.dma_start(tabs["tab_cs"][:], D["tab_cs"].ap())
        sy.dma_start(tabs["tab_gcn"][:], D["tab_gcn"].ap())

        def load_late_tabs():
            sy.dma_start(m01_sb[:], D["m01"].ap())
            for nm in ("tab_bq", "tab_bk", "tab_bo", "tab_b2", "tab_b1",
                       "tab_g1", "tab_g2"):
                sy.dma_start(tabs[nm][:], D[nm].ap())

        # x master (f32r) + bf16 mirror, feature-major [128, 680] x 4
        xb = [[persist.tile([128, L], F32R, name=f"xb_{b}_{d}")
               for d in range(4)] for b in range(LB)]
        x16 = [[persist.tile([128, L], BF16, name=f"x16_{b}_{d}")
                for d in range(4)] for b in range(LB)]
        kT = [[persist.tile([128, L], BF16, name=f"kT_{b}_{d}")
               for d in range(4)] for b in range(LB)]
        vplus = [[persist.tile([128, 520], BF16, name=f"vp_{b}_{t}")
                  for t in range(6)] for b in range(LB)]
        for b in range(LB):
            for t in range(6):
                ov = vplus[b][t][:].rearrange("p (h j) -> p h j", h=8)
                v.memset(ov[:, :, 64:65], 1.0)
        # exp'd scores, packed group layout, double-buffered by head parity
        se_t = {}
        for hf in range(2):
            for par in range(2):
                for g in range(ng[hf]):
                    gw = grp_w[(hf, g)]
                    se_t[(hf, par, g)] = persist.tile(
                        [128, gw], BF16, name=f"se{hf}{par}{g}")

        lay_box = {}

        def row_t(name):
            return small.tile([1, HW], F32R, tag="rows", bufs=4, name=name)

        def rowf_t(name):
            return small.tile([1, HW], F32, tag="rowsf", bufs=3, name=name)

        def sq_t(name):
            return small.tile([128, HW], BF16, tag="sq", bufs=8, name=name)

        def t1_t(name):
            return small.tile([128, HW], F32, tag="t1", bufs=2, name=name)

        def prb_t(name):
            return small.tile([64, HW], F32R, tag="prb", bufs=2, name=name)

        # ---- layer weights: qkvo/bv double-buffered; w1/w2 single-buffered
        # (their load emission points are constrained by tile dep order) ----
        W = {l: {} for l in range(NL)}

        def load_qkvo(l):
            for nm, dram in (("wq", D["wq"]), ("wk", D["wk"]),
                             ("wv", D["wv"]), ("wo", D["wo"])):
                ts = []
                for k in range(4):
                    t = wpool.tile([128, DM], BF16, tag=nm, bufs=8,
                                   name=f"{nm}{l}_{k}")
                    sy.dma_start(t[:], dram.ap()[l, k * 128:(k + 1) * 128, :])
                    ts.append(t)
                W[l][nm] = ts
            bv = wpool.tile([128, DM], BF16, tag="bv", bufs=2, name=f"bv{l}")
            sy.dma_start(bv[:], D["bvB"].ap()[l])
            W[l]["bv"] = bv

        def load_w1(l):
            ts = []
            for k in range(4):
                t = wpool.tile([128, DFF], BF16, tag="w1", bufs=4,
                               name=f"w1{l}_{k}")
                sy.dma_start(t[:], D["w1"].ap()[l, k * 128:(k + 1) * 128, :])
                ts.append(t)
            W[l]["w1"] = ts

        def load_w2(l):
            ts = []
            for m in range(16):
                t = wpool.tile([128, DM], BF16, tag="w2", bufs=16,
                               name=f"w2{l}_{m}")
                sy.dma_start(t[:], D["w2"].ap()[l, m * 128:(m + 1) * 128, :])
                ts.append(t)
            W[l]["w2"] = ts

        # ---------------- fused LayerNorm over partition dim ----------------
        def g_ln(src, ones_c, g_col, dstf, dstb, stats_bank, r_bank, nm):
            """dst_d = ((src_d - mean) * g_d) * rstd  (bias folded elsewhere).

            src: 4 APs [128, HW]; ones_c matches src dtype; writes f32r
            master dstf + bf16 mirror dstb.
            """
            for d in range(4):
                te.matmul(stats_bank[0:1, 0:HW], ones_c[:], src[d],
                          start=(d == 0), stop=(d == 3))
            mrow = row_t(f"mr_{nm}")
            v.tensor_scalar(mrow[:], stats_bank[0:1, 0:HW], 1.0 / 512.0,
                            None, AO.mult)
            yield
            sqs = []
            for d in range(4):
                s = sq_t(f"sq_{nm}_{d}")
                gp.tensor_tensor(s[:], src[d], src[d], AO.mult)
                sqs.append(s)
            yield
            for d in range(4):
                te.matmul(stats_bank[0:1, 0:HW], ones128b[:], sqs[d][:],
                          start=(d == 0), stop=(d == 3))
            m2 = rowf_t(f"m2_{nm}")
            gp.tensor_tensor(m2[:], mrow[:], mrow[:], AO.mult)
            vrow = rowf_t(f"vr_{nm}")
            v.scalar_tensor_tensor(vrow[:], stats_bank[0:1, 0:HW],
                                   1.0 / 512.0, m2[:], AO.mult, AO.subtract)
            lnv = rowf_t(f"lv_{nm}")
            sc.activation(lnv[:], vrow[:], Act.Ln, bias=epsT[:])
            rrow = row_t(f"rr_{nm}")
            sc.activation(rrow[:], lnv[:], Act.Exp, scale=-0.5)
            yield
            # M/R broadcast + all tail reads must be ONE emission chunk:
            # interleaved streams may overwrite shared psum banks between
            # chunks (emission order defines tile deps).
            te.matmul(stats_bank[0:128, 0:HW], ones1[:], mrow[:],
                      start=True, stop=True)
            te.matmul(r_bank[0:128, 0:HW], ones1[:], rrow[:],
                      start=True, stop=True)
            for d in range(4):
                t1 = t1_t(f"t1_{nm}_{d}")
                v.tensor_tensor(t1[:], src[d], stats_bank[0:128, 0:HW],
                                AO.subtract)
                v.scalar_tensor_tensor(dstf[d], t1[:], g_col[:, d:d + 1],
                                       r_bank[0:128, 0:HW], AO.mult, AO.mult)
                gp.tensor_copy(dstb[d], dstf[d])
            yield

        # ---------------- stage generators ----------------
        def g_kv(l, b):
            wk_sb, wv_sb, bv_sb = W[l]["wk"], W[l]["wv"], W[l]["bv"]
            t_bk = tabs["tab_bk"]
            for hf in range(2):
                hs = slice(hf * HW, (hf + 1) * HW)
                for d in range(4):
                    pk = qb_next()
                    for k in range(4):
                        te.matmul(pk[:, 0:HW],
                                  wk_sb[k][:, d * 128:(d + 1) * 128],
                                  x16[b][k][:, hs],
                                  start=(k == 0), stop=(k == 3))
                    sc.activation(kT[b][d][:, hs], pk[:, 0:HW], Act.Identity,
                                  bias=t_bk[:, l * 4 + d:l * 4 + d + 1])
                    yield
            for t in range(6):
                a, bb = KT[t]
                w = bb - a
                pv = pob_next()
                for k in range(4):
                    te.matmul(pv[0:w, 0:DM], x16[b][k][:, a:bb], wv_sb[k][:],
                              start=(k == 0), stop=(k == 3))
                ov = vplus[b][t][0:w].rearrange("p (h j) -> p h j", h=8)
                pvv = pv[0:w, 0:DM].rearrange("p (h j) -> p h j", h=8)
                bvv = bv_sb[0:w].rearrange("p (h j) -> p h j", h=8)
                v.tensor_tensor(ov[:, :, 0:64], pvv, bvv, AO.add)
                yield

        def g_attn(l, b, hf, carry_ln=False):
            wq_sb, wo_sb = W[l]["wq"], W[l]["wo"]
            t_bq, t_bo = tabs["tab_bq"], tabs["tab_bo"]
            hs = slice(hf * HW, (hf + 1) * HW)
            qTh = []
            for d in range(4):
                pq = qb_next()
                for k in range(4):
                    te.matmul(pq[:, 0:HW],
                              wq_sb[k][:, d * 128:(d + 1) * 128],
                              x16[b][k][:, hs], start=(k == 0), stop=(k == 3))
                qt = lay_box["pool"].tile([128, HW], BF16, tag="qTh", bufs=8,
                                name=f"qT{l}{b}{hf}_{d}")
                sc.activation(qt[:], pq[:, 0:HW], Act.Identity,
                              bias=t_bq[:, l * 4 + d:l * 4 + d + 1])
                qTh.append(qt)
                yield
            oTh = [lay_box["pool"].tile([128, HW], BF16, tag="oTh", bufs=8,
                              name=f"oT{l}{b}{hf}_{d}") for d in range(4)]
            swin = [(r, KT[r][0], KT[r][1] - KT[r][0], win[(r, hf)])
                    for r in range(6) if (r, hf) in win]
            pend = [None]
            for h in range(NH):
                d4, r64 = h // 2, (h % 2) * 64
                par = h % 2
                for g in range(ng[hf]):
                    ps = sb[g]
                    for (r, a, kp, wl) in swin:
                        for (q0, w, gg, poff) in wl:
                            if gg != g:
                                continue
                            te.matmul(ps[0:kp, poff:poff + w],
                                      kT[b][d4][r64:r64 + 64, a:a + kp],
                                      qTh[d4][r64:r64 + 64, q0:q0 + w],
                                      start=True, stop=True)
                    gw = grp_w[(hf, g)]
                    st = se_t[(hf, par, g)]
                    sc.activation(st[:, 0:gw], ps[0:128, 0:gw], Act.Exp)
                    mo = m01offs[(hf, g)]
                    v.tensor_tensor(st[:, 0:gw], st[:, 0:gw],
                                    m01_sb[:, mo:mo + gw], AO.mult)
                    yield
                po = pob_next()
                for (q0, wseg, contribs) in segs[hf]:
                    n = len(contribs)
                    for i, (r, g, poff) in enumerate(contribs):
                        kp = KT[r][1] - KT[r][0]
                        te.matmul(po[0:65, q0:q0 + wseg],
                                  vplus[b][r][0:kp, h * 65:h * 65 + 65],
                                  se_t[(hf, par, g)][0:kp, poff:poff + wseg],
                                  start=(i == 0), stop=(i == n - 1))
                # normalize pipelined one head back: recip+bcast now, the
                # po*prb mult next chunk (prb ready by then; avoids DVE
                # head-of-line stall on the Pool broadcast).
                rr = small.tile([1, HW], F32R, tag="rr", bufs=2,
                                name=f"rc{l}{b}{hf}{h}")
                v.reciprocal(rr[:], po[64:65, 0:HW])
                pb = prb_t(f"pb{l}{b}{hf}{h}")
                gp.partition_broadcast(pb[:], rr[:])
                if pend[0] is not None:
                    po_, pb_, h_ = pend[0]
                    v.tensor_tensor(oTh[h_ // 2][(h_ % 2) * 64:
                                                 (h_ % 2) * 64 + 64, 0:HW],
                                    po_[0:64, 0:HW], pb_[0:64, :], AO.mult)
                pend[0] = (po, pb, h)
                yield
            if pend[0] is not None:
                po_, pb_, h_ = pend[0]
                v.tensor_tensor(oTh[h_ // 2][(h_ % 2) * 64:
                                             (h_ % 2) * 64 + 64, 0:HW],
                                po_[0:64, 0:HW], pb_[0:64, :], AO.mult)
                pend[0] = None
            resid = []
            for d in range(4):
                pa = qb_next()
                for k in range(4):
                    te.matmul(pa[:, 0:HW],
                              wo_sb[k][:, d * 128:(d + 1) * 128],
                              oTh[k][:], start=(k == 0), stop=(k == 3))
                rs = lay_box["pool"].tile([128, HW], F32R, tag="resid", bufs=8,
                                name=f"rs{l}{b}{hf}_{d}")
                v.scalar_tensor_tensor(rs[:], pa[:, 0:HW],
                                       t_bo[:, l * 4 + d:l * 4 + d + 1],
                                       xb[b][d][:, hs], AO.add, AO.add)
                resid.append(rs)
                yield
            def mk_ln1(stats_bank, r_bank):
                return g_ln([r[:] for r in resid], ones128f,
                            tabs["tab_g1"][:, l * 4:l * 4 + 4],
                            [xb[b][d][:, hs] for d in range(4)],
                            [x16[b][d][:, hs] for d in range(4)],
                            stats_bank, r_bank, f"l1_{l}{b}{hf}")
            if carry_ln:
                carry.append(mk_ln1)
            else:
                yield from mk_ln1(lnb, sb[0])

        def g_f1(l, b, hf, hT_out):
            w1_sb = W[l]["w1"]
            t_b1 = tabs["tab_b1"]
            hs = slice(hf * HW, (hf + 1) * HW)
            for m in range(16):
                ph = qb_next()
                for k in range(4):
                    te.matmul(ph[:, 0:HW],
                              w1_sb[k][:, m * 128:(m + 1) * 128],
                              x16[b][k][:, hs], start=(k == 0), stop=(k == 3))
                t = lay_box["pool"].tile([128, HW], BF16, tag="hT", bufs=34,
                               name=f"hT{l}{b}{hf}_{m}")
                sc.activation(t[:], ph[:, 0:HW], Act.Gelu,
                              bias=t_b1[:, l * 16 + m:l * 16 + m + 1])
                hT_out.append(t)
                if m % 2 == 1:
                    yield

        def g_f2(l, b, hf, hT, py_bank=None, r_bank=None, carry_ln=False):
            w2_sb = W[l]["w2"]
            t_b2 = tabs["tab_b2"]
            py_bank = ffnb if py_bank is None else py_bank
            r_bank = sb[1] if r_bank is None else r_bank
            hs = slice(hf * HW, (hf + 1) * HW)
            resid = []
            for d in range(4):
                for m in range(16):
                    te.matmul(py_bank[:, 0:HW],
                              w2_sb[m][:, d * 128:(d + 1) * 128],
                              hT[m][:], start=(m == 0), stop=(m == 15))
                    if m % 2 == 1 and m < 15:
                        yield
                rs = lay_box["pool"].tile([128, HW], F32R, tag="resid", bufs=8,
                                name=f"r2{l}{b}{hf}_{d}")
                v.scalar_tensor_tensor(rs[:], py_bank[:, 0:HW],
                                       t_b2[:, l * 4 + d:l * 4 + d + 1],
                                       xb[b][d][:, hs], AO.add, AO.add)
                resid.append(rs)
                yield
            def mk_ln2(stats_bank, r_bank):
                return g_ln([r[:] for r in resid], ones128f,
                            tabs["tab_g2"][:, l * 4:l * 4 + 4],
                            [xb[b][d][:, hs] for d in range(4)],
                            [x16[b][d][:, hs] for d in range(4)],
                            stats_bank, r_bank, f"l2_{l}{b}{hf}")
            if carry_ln:
                carry.append(mk_ln2)
            else:
                yield from mk_ln2(py_bank, r_bank)

        # ---------------- CSCM: embedding + conv pyramid + LN ----------------
        def run_cscm(cp, after_embed=None):
            t_cs = tabs["tab_cs"]
            xs_t = [cp.tile([25, 2 * SEQ], F32R, tag="x25", bufs=2,
                            name=f"x25p_{j}") for j in range(2)]
            wp_sb = cp.tile([25, DM + 128], F32R, name="wp_sb")
            sy.dma_start(xs_t[0][:], D["x25p"].ap()[0])
            sy.dma_start(wp_sb[:], D["wp"].ap())
            dcorr_sb = cp.tile([128, SEQ], F32, name="dcorr_sb")
            sy.dma_start(dcorr_sb[:], D["dcorr"].ap())
            sy.dma_start(xs_t[1][:], D["x25p"].ap()[1])
            w25_sb = wp_sb[:, 0:DM]
            w25d_sb = wp_sb[:, DM:DM + 128]
            cembP = cp.tile([128, 4 * SEQ], BF16, name="cembP")
            sy.dma_start(cembP[:], D["cembP"].ap())
            cemb_sb = [cembP[:, d * SEQ:(d + 1) * SEQ] for d in range(4)]
            ckP = cp.tile([128, 1536], BF16, name="ckP")
            sy.dma_start(ckP[:], D["ckP"].ap())
            convk_sb = [[ckP[:, (i * 4 + w) * 128:(i * 4 + w + 1) * 128]
                         for w in range(4)] for i in range(3)]
            upw_sb = cp.tile([128, DM], BF16, name="upw_sb")
            sy.dma_start(upw_sb[:], D["upw"].ap())

            xcat = [[cp.tile([128, L], BF16, name=f"xc{b}_{d}")
                     for d in range(4)] for b in range(LB)]
            c1pre = cp.tile([128, NB * 128], BF16, name="c1pre")
            c2pre = cp.tile([128, NB * 32], BF16, name="c2pre")
            c3pre = cp.tile([128, NB * 8], BF16, name="c3pre")
            c1e = cp.tile([128, NB * 128], BF16, name="c1e")
            c2e = cp.tile([128, NB * 32], BF16, name="c2e")
            c3e = cp.tile([128, NB * 8], BF16, name="c3e")

            # embedding + down-proj + conv1, pipelined per batch
            for b in range(NB):
                j = b // 2
                if b % 2 == 0:
                    if j < 2:
                        xsp = xs_t[j]
                    else:
                        xsp = cp.tile([25, 2 * SEQ], F32R, tag="x25", bufs=2,
                                      name=f"x25p_{j}")
                        sy.dma_start(xsp[:], D["x25p"].ap()[j])
                    cur_pair = xsp
                xs = cur_pair[:, (b % 2) * SEQ:(b % 2 + 1) * SEQ]
                pd = qb_next()
                te.matmul(pd[:, 0:SEQ], w25d_sb, xs,
                          start=True, stop=True)
                db = cp.tile([128, SEQ], BF16, tag="dall", bufs=3,
                             name=f"dall{b}")
                v.tensor_tensor(db[:], pd[:, 0:SEQ], dcorr_sb[:], AO.add)
                if b < LB:
                    for d in range(4):
                        pe = pob_next()
                        te.matmul(pe[:, 0:SEQ],
                                  w25_sb[:, d * 128:(d + 1) * 128], xs,
                                  start=True, stop=True)
                        v.tensor_tensor(xcat[b][d][:, 0:SEQ], pe[:, 0:SEQ],
                                        cemb_sb[d], AO.add)
                pc = qb_next()
                for w in range(4):
                    te.matmul(pc[:, 0:128], convk_sb[0][w],
                              db[:, w:SEQ:4], start=(w == 0), stop=(w == 3))
                sc.activation(c1pre[:, b * 128:(b + 1) * 128], pc[:, 0:128],
                              Act.Identity, bias=t_cs[:, 0:1])

            if after_embed is not None:
                after_embed()

            def bn_elu(pre, eo, n_elem, lvl, nch):
                """eo = ELU((pre - mu)/sqrt(var+eps) * bng + bnb), training BN."""
                s1 = cp.tile([128, 1], F32, tag="r1", bufs=6, name=f"s1_{lvl}")
                v.tensor_reduce(s1[:], pre[:], mybir.AxisListType.X,
                                op=AO.add)
                cw = n_elem // nch
                ssq4 = cp.tile([128, nch], F32, tag="r4", bufs=2,
                               name=f"sq4_{lvl}")
                junk = cp.tile([128, cw], BF16, tag="zc", bufs=2,
                               name=f"jk_{lvl}")
                for c in range(nch):
                    sc.activation(junk[:], pre[:, c * cw:(c + 1) * cw],
                                  Act.Square, accum_out=ssq4[:, c:c + 1])
                ssq = cp.tile([128, 1], F32, tag="r1", bufs=6,
                              name=f"ssq_{lvl}")
                v.tensor_reduce(ssq[:], ssq4[:], mybir.AxisListType.X,
                                op=AO.add)
                mean = cp.tile([128, 1], F32, tag="r1", bufs=6,
                               name=f"mn_{lvl}")
                sc.activation(mean[:], s1[:], Act.Copy, scale=1.0 / n_elem)
                m2 = cp.tile([128, 1], F32, tag="r1", bufs=6, name=f"m2_{lvl}")
                gp.tensor_tensor(m2[:], mean[:], mean[:], AO.mult)
                var = cp.tile([128, 1], F32, tag="r1", bufs=6,
                              name=f"vr_{lvl}")
                v.scalar_tensor_tensor(var[:], ssq[:], 1.0 / n_elem, m2[:],
                                       AO.mult, AO.subtract)
                lnv = cp.tile([128, 1], F32, tag="r1", bufs=6,
                              name=f"lw_{lvl}")
                sc.activation(lnv[:], var[:], Act.Ln, bias=eps128[:])
                rstd = cp.tile([128, 1], F32, tag="r1", bufs=6,
                               name=f"rs_{lvl}")
                sc.activation(rstd[:], lnv[:], Act.Exp, scale=-0.5)
                seff = cp.tile([128, 1], F32, tag="r1", bufs=6,
                               name=f"se_{lvl}")
                gp.tensor_tensor(seff[:], rstd[:], t_cs[:, 3 + lvl:4 + lvl],
                                 AO.mult)
                sm = cp.tile([128, 1], F32, tag="r1", bufs=6, name=f"sm_{lvl}")
                gp.tensor_tensor(sm[:], mean[:], seff[:], AO.mult)
                shift = cp.tile([128, 1], F32, tag="r1", bufs=6,
                                name=f"sh_{lvl}")
                v.scalar_tensor_tensor(shift[:], sm[:], -1.0,
                                       t_cs[:, 6 + lvl:7 + lvl],
                                       AO.mult, AO.add)
                for c in range(nch):
                    cs = slice(c * cw, (c + 1) * cw)
                    z = cp.tile([128, cw], BF16, tag="zc", bufs=2,
                                name=f"z{lvl}_{c}")
                    sc.activation(z[:], pre[:, cs], Act.Identity,
                                  scale=seff[:], bias=shift[:])
                    zm = cp.tile([128, cw], BF16, tag="zm", bufs=2,
                                 name=f"zm{lvl}_{c}")
                    gp.tensor_scalar(zm[:], z[:], 0.0, None, AO.min)
                    ze = cp.tile([128, cw], BF16, tag="zc", bufs=2,
                                 name=f"ze{lvl}_{c}")
                    sc.activation(ze[:], zm[:], Act.Exp)
                    zr = cp.tile([128, cw], BF16, tag="zm", bufs=2,
                                 name=f"zr{lvl}_{c}")
                    v.tensor_scalar(zr[:], z[:], 0.0, None, AO.max)
                    v.scalar_tensor_tensor(eo[:, cs], ze[:], -1.0, zr[:],
                                           AO.add, AO.add)

            bn_elu(c1pre, c1e, NB * 128, 0, 4)
            for b in range(NB):
                pc = qb_next()
                for w in range(4):
                    te.matmul(pc[:, 0:32], convk_sb[1][w],
                              c1e[:, b * 128 + w:b * 128 + 128:4],
                              start=(w == 0), stop=(w == 3))
                sc.activation(c2pre[:, b * 32:(b + 1) * 32], pc[:, 0:32],
                              Act.Identity, bias=t_cs[:, 1:2])
            bn_elu(c2pre, c2e, NB * 32, 1, 2)
            for b in range(NB):
                pc = qb_next()
                for w in range(4):
                    te.matmul(pc[:, 0:8], convk_sb[2][w],
                              c2e[:, b * 32 + w:b * 32 + 32:4],
                              start=(w == 0), stop=(w == 3))
                sc.activation(c3pre[:, b * 8:(b + 1) * 8], pc[:, 0:8],
                              Act.Identity, bias=t_cs[:, 2:3])
            bn_elu(c3pre, c3e, NB * 8, 2, 1)

            for b in range(LB):
                cat = cp.tile([128, 168], BF16, tag="cat", bufs=2,
                              name=f"cat{b}")
                gp.tensor_copy(cat[:, 0:128], c1e[:, b * 128:(b + 1) * 128])
                gp.tensor_copy(cat[:, 128:160], c2e[:, b * 32:(b + 1) * 32])
                gp.tensor_copy(cat[:, 160:168], c3e[:, b * 8:(b + 1) * 8])
                for d in range(4):
                    pu = qb_next()
                    te.matmul(pu[:, 0:168],
                              upw_sb[:, d * 128:(d + 1) * 128], cat[:],
                              start=True, stop=True)
                    sc.activation(xcat[b][d][:, SEQ:L], pu[:, 0:168],
                                  Act.Identity, bias=t_cs[:, 12 + d:13 + d])
            def cn_ln(b, hf):
                hs = slice(hf * HW, (hf + 1) * HW)
                return g_ln([xcat[b][d][:, hs] for d in range(4)],
                            ones128b, tabs["tab_gcn"],
                            [xb[b][d][:, hs] for d in range(4)],
                            [x16[b][d][:, hs] for d in range(4)],
                            lnb if hf == 0 else ffnb,
                            sb[hf], f"cn{b}{hf}")

            _drive(cn_ln(0, 0), cn_ln(0, 1))
            return [cn_ln(1, 0), cn_ln(1, 1)]

        # ---------------- schedule ----------------
        with tc.tile_pool(name="cscm", bufs=1) as cp:
            cn_tail = run_cscm(cp, after_embed=lambda: (load_late_tabs(),
                                                        load_qkvo(0),
                                                        load_w1(0),
                                                        load_w2(0)))
            # overlap batch-1's CSCM LayerNorm with layer-0 KV(b0): KV only
            # needs x16[0], produced by the batch-0 LN above.
            _drive(g_kv(0, 0), *cn_tail)
        lay_box["pool"] = ctx.enter_context(tc.tile_pool(name="lay", bufs=1))
        if "dbg0" in D:
            for d in range(4):
                sy.dma_start(D["dbg0"].ap()[d], x16[0][d][:])
        load_qkvo(1)

        hT = {}
        carry = []  # deferred LN-tail factories from s3/s6 stages

        def spend(bks=None):
            # KV/F1 slots never touch ffnb/lnb/sb banks; ATTN/F2 slots get
            # pob banks (free until the heads phase, ~round 8).
            if bks is None:
                bks = [(ffnb, sb[1]), (lnb, sb[0])]
            gens = [mk(s_, r_) for mk, (s_, r_) in zip(carry, bks)]
            del carry[:]
            return gens

        BK_AT = [(pob[0], pob[1]), (lnb, sb[0])]

        f1p = {}  # pending F1 generators keyed (l, b, hf)

        def f1(l, b, hf):
            hT[(l, b, hf)] = []
            return g_f1(l, b, hf, hT[(l, b, hf)])

        for l in range(NL):
            # s1: KV(b0) + F1(prev, b1, both halves)
            if l > 0:
                t = spend()
                _drive(*t, f1(l - 1, 1, 0), g_kv(l, 0), f1(l - 1, 1, 1),
                       delay=[0] * len(t) + [0, 1, 8])
            # s2/s3: ATTN(b0, hf) + F2(prev, b1, hf)
            if l >= 1:
                load_w1(l)
            _drive(g_f2(l - 1, 1, 0, hT[(l - 1, 1, 0)], carry_ln=True)
                   if l > 0 else None,
                   g_attn(l, 0, 0, carry_ln=True))
            t = spend(BK_AT)
            _drive(*t, g_f2(l - 1, 1, 1, hT[(l - 1, 1, 1)], carry_ln=True)
                   if l > 0 else None,
                   g_attn(l, 0, 1, carry_ln=True),
                   delay=[0] * len(t) + [0, 1])
            if l >= 1:
                load_w2(l)
            if l == 0 and "dbg2" in D:
                for d in range(4):
                    sy.dma_start(D["dbg2"].ap()[d], x16[0][d][:])
            # s4: KV(b1) + F1(l, b0, both)
            t = spend()
            _drive(*t, f1(l, 0, 0), g_kv(l, 1), f1(l, 0, 1),
                   delay=[0] * len(t) + [0, 1, 8])
            # s5/s6: ATTN(b1, hf) + F2(l, b0, hf)
            _drive(g_f2(l, 0, 0, hT[(l, 0, 0)], carry_ln=True),
                   g_attn(l, 1, 0, carry_ln=True))
            t = spend(BK_AT)
            _drive(*t, g_f2(l, 0, 1, hT[(l, 0, 1)], carry_ln=True),
                   g_attn(l, 1, 1, carry_ln=True),
                   delay=[0] * len(t) + [0, 1])
            if l + 2 < NL:
                load_qkvo(l + 2)
            if l == 0 and "dbg1" in D:
                for d in range(4):
                    sy.dma_start(D["dbg1"].ap()[d], x16[0][d][:])
        # tail: F1/F2 for (b1) of last layer; b0 dumps overlap the tail
        t = spend()
        _drive(*t, f1(NL - 1, 1, 0), f1(NL - 1, 1, 1),
               delay=[0] * len(t) + [0, 4])
        for d in range(4):
            sy.dma_start(D["out"].ap()[0, d], xb[0][d][:])
        def dump_after(gen, emit):
            yield from gen
            emit()

        def dump_b1h0():
            for d in range(4):
                sy.dma_start(D["out"].ap()[1, d][:, 0:HW], xb[1][d][:, 0:HW])

        _drive(dump_after(g_f2(NL - 1, 1, 0, hT[(NL - 1, 1, 0)]), dump_b1h0),
               g_f2(NL - 1, 1, 1, hT[(NL - 1, 1, 1)],
                    py_bank=lnb, r_bank=sb[0]))
        for d in range(4):
            sy.dma_start(D["out"].ap()[1, d][:, HW:L], xb[1][d][:, HW:L])


# ======================= host side =======================
_PROG = None
_PLAN = None


def _pos_embed(n, d):
    pos = np.arange(n, dtype=np.float32)[:, None]
    div = np.exp(np.arange(0, d, 2, dtype=np.float32) *
                 (-np.log(10000.0) / d))
    pe = np.zeros((n, d), np.float32)
    pe[:, 0::2] = np.sin(pos * div)
    pe[:, 1::2] = np.cos(pos * div)
    return pe


def kernel(**inputs):
    global _PROG, _PLAN
    import ml_dtypes
    bf16 = ml_dtypes.bfloat16
    inputs = {k: np.asarray(v) for k, v in inputs.items()}
    attn_mask = inputs["attn_mask"]
    if _PROG is None:
        win, grp_w, segs = _attn_plan(attn_mask)
        m01, m01offs, m01w, ng = _m01_pack(attn_mask, win, grp_w)
        _PLAN = (win, grp_w, segs, m01, m01offs, ng)
        _PROG = _build_program((win, grp_w, segs, m01w, m01offs, ng))
    win, grp_w, segs, m01, m01offs, ng = _PLAN
    nc = _PROG

    f32 = np.float32
    x_enc = inputs["x_enc"].astype(f32)
    x_mark = inputs["x_mark_enc"].astype(f32)
    tok = inputs["tok_kernel"].astype(f32)

    X25 = np.concatenate([np.roll(x_enc, 1, axis=1), x_enc,
                          np.roll(x_enc, -1, axis=1), x_mark], axis=2)
    X25T = np.ascontiguousarray(X25.transpose(0, 2, 1))  # [B,25,512]
    W25 = np.concatenate([tok[0], tok[1], tok[2], inputs["mark_W"]], axis=0)
    Cemb = _pos_embed(SEQ, DM) + inputs["mark_b"]        # [512tok, 512f]
    CembT = np.ascontiguousarray(Cemb.T).reshape(4, 128, SEQ)
    down_W = inputs["down_W"].astype(f32)
    W25D = W25 @ down_W                                   # [25, 128]
    Dcorr = down_W.T @ Cemb.T + inputs["down_b"][:, None]  # [128, 512tok]

    # LN bias folding: x~ = LN(resid)*g (bias dropped); consumers add W^T b.
    b_prev = [inputs["cn_b"].astype(f32)] + \
             [inputs["ln2_b"][i].astype(f32) for i in range(NL - 1)]
    Wq, Wk, Wv, Wo = (inputs[k].astype(f32) for k in ("Wq", "Wk", "Wv", "Wo"))
    W1, W2 = inputs["W1"].astype(f32), inputs["W2"].astype(f32)
    bq = np.stack([(inputs["bq"][l] + b_prev[l] @ Wq[l]) / 8.0
                   for l in range(NL)])
    bk = np.stack([inputs["bk"][l] + b_prev[l] @ Wk[l] for l in range(NL)])
    bv = np.stack([inputs["bv"][l] + b_prev[l] @ Wv[l] for l in range(NL)])
    b1 = np.stack([inputs["b1"][l] + inputs["ln1_b"][l] @ W1[l]
                   for l in range(NL)])
    bo = np.stack([inputs["bo"][l] + b_prev[l] for l in range(NL)])
    b2 = np.stack([inputs["b2"][l] + inputs["ln1_b"][l] for l in range(NL)])

    cs = np.zeros((128, 16), f32)
    for i in range(3):
        cs[:, 0 + i] = inputs["conv_b"][i]
        cs[:, 3 + i] = inputs["bn_g"][i]
        cs[:, 6 + i] = inputs["bn_b"][i]
    cs[:, 12:16] = inputs["up_b"].reshape(4, 128).T

    ck = inputs["conv_K"].astype(f32)
    ckP = np.zeros((128, 1536), f32)
    for i in range(3):
        for w in range(4):
            ckP[:, (i * 4 + w) * 128:(i * 4 + w + 1) * 128] = ck[i, w]
    com = dict(
        wp=np.concatenate([W25, W25D], axis=1),
        cembP=np.ascontiguousarray(
            CembT.transpose(1, 0, 2).reshape(128, 4 * SEQ)).astype(bf16),
        dcorr=Dcorr,
        ckP=ckP.astype(bf16),
        upw=inputs["up_W"].astype(bf16),
        m01=(m01.astype(bf16)),
        wq=(Wq / 8.0).astype(bf16), wk=Wk.astype(bf16),
        wv=Wv.astype(bf16), wo=Wo.astype(bf16),
        w1=W1.astype(bf16), w2=W2.astype(bf16),
        bvB=np.broadcast_to(bv[:, None, :], (NL, 128, DM)).astype(bf16),
        tab_bq=bq.reshape(NL * 4, 128).T.copy(),
        tab_bk=bk.reshape(NL * 4, 128).T.copy(),
        tab_bo=bo.reshape(NL * 4, 128).T.copy(),
        tab_b2=b2.reshape(NL * 4, 128).T.copy(),
        tab_b1=b1.reshape(NL * 16, 128).T.copy(),
        tab_g1=inputs["ln1_g"].reshape(NL * 4, 128).T.astype(f32).copy(),
        tab_g2=inputs["ln2_g"].reshape(NL * 4, 128).T.astype(f32).copy(),
        tab_gcn=inputs["cn_g"].reshape(4, 128).T.astype(f32).copy(),
        tab_cs=cs,
        ones128_d=np.ones((128, 1), f32),
        ones1_d=np.ones((1, 128), f32),
        zrow_d=np.zeros((1, 512), f32),
    )
    for k in list(com):
        if com[k].dtype != bf16:
            com[k] = np.ascontiguousarray(com[k], f32)
        else:
            com[k] = np.ascontiguousarray(com[k])

    in_maps = []
    for c in range(N_CORES):
        order = [2 * c, 2 * c + 1] + \
                [i for i in range(NB) if i not in (2 * c, 2 * c + 1)]
        m = dict(com)
        xo = X25T[order]  # [16, 25, 512]
        m["x25p"] = np.ascontiguousarray(
            xo.reshape(8, 2, 25, SEQ).transpose(0, 2, 1, 3)
            .reshape(8, 25, 2 * SEQ))
        in_maps.append(m)

    res = run_bass_kernel_spmd(nc, in_maps, core_ids=list(range(N_CORES)))
    globals()["_LAST_RES"] = res

    X = np.empty((NB, L, DM), f32)
    for c in range(N_CORES):
        o = np.asarray(res.results[c]["out"]).astype(f32)  # [2,4,128,680]
        for j in range(LB):
            X[2 * c + j] = o[j].reshape(DM, L).T
    X += inputs["ln2_b"][NL - 1].astype(f32)[None, None, :]
    gidx = np.asarray(inputs["gather_idx"]).astype(np.int64)
    out = X[:, gidx, :].reshape(NB, SEQ, NH * 4 * 64)
    return out.astype(f32)
